# revision 65
# baseline (speedup 1.0000x reference)
"""Mamba block (add+RMSNorm -> in_proj -> causal conv1d -> SSM scan -> out_proj)
on 8 Trainium2 NeuronCores.

Sharding: 8-way tensor-parallel over d_inner (256 channels per core); every
core processes all 4096 tokens (the scan recurrence stays on-core).
Cross-core communication: two small bf16 AllReduces for the x_proj partial
sums (one per batch), and one bf16 AllToAll of the gated SSM output so each
core runs out_proj for 256 tokens of each batch with the full d_inner
contraction.

Engine split: selective-scan ops on Pool (gpsimd), elementwise mults on DVE
(bf16 2x mode), exps/softplus/drains on Activation, GEMMs + y-reduction on PE.
norm_weight/norm_bias/A_log/D_param are folded host-side.
"""

import sys

for _p in ("/opt/trn_rl_repo", "/root/.axon_site/_ro/trn_rl_repo"):
    if _p not in sys.path:
        sys.path.insert(0, _p)

import numpy as np
from contextlib import ExitStack

import concourse.bacc as bacc
import concourse.mybir as mybir
import concourse.tile as tile
from concourse.masks import make_identity

F32 = mybir.dt.float32
BF16 = mybir.dt.bfloat16
AF = mybir.ActivationFunctionType
OP = mybir.AluOpType

# problem shapes (hardcoded)
DIM = 1024
D_INNER = 2048
D_STATE = 16
D_CONV = 4
DT_RANK = 64
BATCH = 2
SEQ = 2048
EPS = 1e-5

N_CORES = 8
DG = D_INNER // N_CORES          # 256 channels per core
NDT = DG // 128                  # 2 d-tiles per core
NKT = DIM // 128                 # 8 k-tiles over d_model
QTOK = (BATCH * SEQ) // N_CORES  # 512 tokens output slice per core
HTOK = QTOK // BATCH             # 256 tokens per batch per core
GROUPS = [list(range(N_CORES))]
LH = SEQ // 2                    # L-half for the norm/in_proj stage
NX = DT_RANK + 2 * D_STATE       # 96

_cache = {}
SIM_NO_COLLECTIVES = False


def _build():
    if "nc" in _cache:
        return _cache["nc"]

    nc = bacc.Bacc("TRN2", target_bir_lowering=False, debug=False,
                   num_devices=N_CORES)

    dram_in = lambda n, s, d=F32: nc.declare_dram_parameter(n, list(s), d, isOutput=False)
    dram_out = lambda n, s, d=F32: nc.declare_dram_parameter(n, list(s), d, isOutput=True)

    # ---- inputs (per-core values, same shapes on every core) ----
    rs_T = dram_in("rs_T", (BATCH, DIM, SEQ), BF16)        # hid+res, host-added
    inproj_wT = dram_in("inproj_wT", (DIM, 2 * DG), BF16)  # norm_w folded
    inproj_b = dram_in("inproj_b", (2 * DG, 1))            # W @ norm_bias
    conv_diag = dram_in("conv_diag", (D_CONV * NDT * 128, 128), BF16)
    conv_b = dram_in("conv_b", (DG, 1))
    xproj_wT = dram_in("xproj_wT", (DG, NX), BF16)
    dtproj_wT = dram_in("dtproj_wT", (DT_RANK, DG), BF16)
    dtproj_b = dram_in("dtproj_b", (DG, 1))
    A_g = dram_in("A_g", (DG, D_STATE))                    # -exp(A_log), host
    d_diag = dram_in("d_diag", (NDT * 128, 128), BF16)     # diag(D_param)
    outproj_wT = dram_in("outproj_wT", (D_INNER, DIM), BF16)  # replicated

    # ---- outputs ----
    out_q = dram_out("out_q", (QTOK, DIM))                 # [tok, d_model]

    # ---- internal DRAM for collectives ----
    ar_in = [nc.dram_tensor(f"ar_in{b}", [NX, SEQ], BF16) for b in range(BATCH)]
    ar_out = [nc.dram_tensor(f"ar_out{b}", [NX, SEQ], BF16, addr_space="Shared")
              for b in range(BATCH)]
    a2a_in = [nc.dram_tensor(f"a2a_in{b}", [N_CORES, DG, HTOK], BF16)
              for b in range(BATCH)]
    a2a_out = [nc.dram_tensor(f"a2a_out{b}", [N_CORES, DG, HTOK], BF16)
               for b in range(BATCH)]

    with tile.TileContext(nc) as tc, ExitStack() as ctx:
        wp = ctx.enter_context(tc.tile_pool(name="weights", bufs=1))

        # resident weights
        w_inproj = wp.tile([128, NKT * 2 * DG], BF16)      # 8 ktiles side by side
        nc.sync.dma_start(w_inproj[:].rearrange("p (k m) -> p k m", k=NKT),
                          inproj_wT[:].rearrange("(k p) m -> p k m", p=128))
        w_diag = wp.tile([128, D_CONV * NDT * 128], BF16)
        nc.sync.dma_start(w_diag[:].rearrange("p (j m) -> p j m", j=D_CONV * NDT),
                          conv_diag[:].rearrange("(j p) m -> p j m", p=128))
        w_ddiag = wp.tile([128, NDT * 128], BF16)
        nc.sync.dma_start(w_ddiag[:].rearrange("p (j m) -> p j m", j=NDT),
                          d_diag[:].rearrange("(j p) m -> p j m", p=128))
        w_xproj = wp.tile([128, NDT * NX], BF16)
        nc.sync.dma_start(w_xproj[:].rearrange("p (k m) -> p k m", k=NDT),
                          xproj_wT[:].rearrange("(k p) m -> p k m", p=128))
        w_dtproj = wp.tile([64, DG], BF16)
        nc.sync.dma_start(w_dtproj[:], dtproj_wT[:])
        bias_sb = wp.tile([128, 2 * DG // 128], F32)
        nc.sync.dma_start(bias_sb[:], inproj_b[:].rearrange("(k p) o -> p k o", p=128).squeeze(-1))
        c_cb = wp.tile([128, NDT], F32)
        nc.sync.dma_start(c_cb[:], conv_b[:].rearrange("(k p) o -> p k o", p=128).squeeze(-1))
        c_dtb = wp.tile([128, NDT], F32)
        nc.sync.dma_start(c_dtb[:], dtproj_b[:].rearrange("(k p) o -> p k o", p=128).squeeze(-1))
        c_A = wp.tile([128, NDT * D_STATE], F32)
        nc.sync.dma_start(c_A[:].rearrange("p (k n) -> p k n", k=NDT),
                          A_g[:].rearrange("(k p) n -> p k n", p=128))
        ones1 = wp.tile([1, 128], F32)
        nc.vector.memset(ones1[:], 1.0)
        ones128_bf = wp.tile([128, 1], BF16)
        nc.vector.memset(ones128_bf[:], 1.0)
        eps_t = wp.tile([1, 1], F32)
        nc.vector.memset(eps_t[:], EPS)
        iden_bf = wp.tile([128, 128], BF16)
        make_identity(nc, iden_bf[:])
        w_outproj = wp.tile([128, (D_INNER // 128) * DIM], BF16)

        # persistent activations (both batches)
        ap_ = ctx.enter_context(tc.tile_pool(name="acts", bufs=1))
        xi = [[ap_.tile([128, SEQ], BF16, tag=f"xi{b}{d}", name=f"xi{b}{d}") for d in range(NDT)]
              for b in range(BATCH)]
        siluz = [[ap_.tile([128, SEQ], BF16, tag=f"sz{b}{d}", name=f"sz{b}{d}") for d in range(NDT)]
                 for b in range(BATCH)]
        dt_t = [[ap_.tile([128, SEQ], BF16, tag=f"dt{b}{d}", name=f"dt{b}{d}") for d in range(NDT)]
                for b in range(BATCH)]
        du = [[ap_.tile([128, SEQ], BF16, tag=f"du{b}{d}", name=f"du{b}{d}") for d in range(NDT)]
              for b in range(BATCH)]
        xi_pre = xi        # conv runs in place (right-to-left chunks)

        # ---------- front + dt helpers (shared 4-bank psum pool) ----------
        fp_pool = tc.tile_pool(name="front", bufs=1)
        fp = fp_pool.__enter__()
        fps_pool = tc.tile_pool(name="frontps", bufs=1, space="PSUM")
        fps = fps_pool.__enter__()

        def front_batch(b):
            for lh in range(2):
                sl = slice(lh * LH, (lh + 1) * LH)
                res_t = [fp.tile([128, LH], BF16, tag=f"res{k}", name=f"res{k}",
                                 bufs=2) for k in range(NKT)]
                for kt in range(NKT):
                    nc.sync.dma_start(res_t[kt][:], rs_T[b, kt * 128:(kt + 1) * 128, sl])
                ssq_sb = fp.tile([1, LH], F32, tag="ssqsb", name="ssqsb")
                for lc in range(LH // 512):
                    sc = fps.tile([1, 512], F32, tag="ssq", name="ssq", bufs=1)
                    for kt in range(NKT):
                        sq = fp.tile([128, 512], BF16, tag="sq", name="sq", bufs=2)
                        nc.vector.tensor_tensor(sq[:],
                                                res_t[kt][:, lc * 512:(lc + 1) * 512],
                                                res_t[kt][:, lc * 512:(lc + 1) * 512],
                                                OP.mult)
                        nc.tensor.matmul(sc[:], ones128_bf[:], sq[:],
                                         start=(kt == 0), stop=(kt == NKT - 1))
                    nc.scalar.activation(ssq_sb[:, lc * 512:(lc + 1) * 512], sc[:],
                                         AF.Copy)
                lnv = fp.tile([1, LH], F32, tag="lnv", name="lnv")
                nc.scalar.activation(lnv[:], ssq_sb[:], AF.Ln, bias=eps_t[:],
                                     scale=1.0 / DIM)
                rstd = fp.tile([1, LH], F32, tag="rstd", name="rstd")
                nc.scalar.activation(rstd[:], lnv[:], AF.Exp, scale=-0.5)
                rrep_sb = fp.tile([128, LH], BF16, tag="rrepsb", name="rrepsb")
                for lc in range(LH // 512):
                    rr = fps.tile([128, 512], F32, tag="rrep", name="rrep", bufs=1)
                    nc.tensor.matmul(rr[:], ones1[:], rstd[:, lc * 512:(lc + 1) * 512],
                                     start=True, stop=True)
                    nc.vector.tensor_scalar_mul(rrep_sb[:, lc * 512:(lc + 1) * 512],
                                                rr[:], 1.0)
                # in_proj; mt-outer groups same-table drains; lc-inner reuses ldweights
                for mt in range(2 * DG // 128):
                    pts = [fps.tile([128, 512], F32, tag=f"mm{lc}", name="mm", bufs=1)
                           for lc in range(LH // 512)]
                    for kt in range(NKT):
                        for lc in range(LH // 512):
                            nc.tensor.matmul(
                                pts[lc][:],
                                w_inproj[:, (kt * 2 * DG) + mt * 128:
                                         (kt * 2 * DG) + (mt + 1) * 128],
                                res_t[kt][:, lc * 512:(lc + 1) * 512],
                                start=(kt == 0), stop=(kt == NKT - 1))
                    for lc in range(LH // 512):
                        col = slice(lh * LH + lc * 512, lh * LH + (lc + 1) * 512)
                        tmp = fp.tile([128, 512], BF16, tag="dtmp", name="dtmp", bufs=2)
                        nc.vector.tensor_tensor(tmp[:], pts[lc][:],
                                                rrep_sb[:, lc * 512:(lc + 1) * 512],
                                                OP.mult)
                        if mt < NDT:
                            if b == 0:
                                nc.vector.tensor_scalar_add(xi_pre[b][mt][:, col],
                                                            tmp[:],
                                                            bias_sb[:, mt:mt + 1])
                            else:
                                nc.scalar.activation(xi_pre[b][mt][:, col], tmp[:],
                                                     AF.Identity,
                                                     bias=bias_sb[:, mt:mt + 1])
                        else:
                            nc.scalar.activation(siluz[b][mt - NDT][:, col], tmp[:],
                                                 AF.Silu,
                                                 bias=bias_sb[:, mt:mt + 1])
            # conv (in place, right-to-left) + silu
            for d in range(NDT):
                for lp in (1, 0):
                    pts = [fps.tile([128, 512], F32, tag=f"mm{i}", name="conv",
                                    bufs=1) for i in range(2)]
                    for j in range(D_CONV):
                        shift = D_CONV - 1 - j
                        for i in (1, 0):
                            base = (lp * 2 + i) * 512
                            lo, hi = base - shift, base + 512 - shift
                            olo = 0
                            if lo < 0:
                                olo, lo = -lo, 0
                            nc.tensor.matmul(
                                pts[i][:, olo:512],
                                w_diag[:, (j * NDT + d) * 128:(j * NDT + d + 1) * 128],
                                xi_pre[b][d][:, lo:hi],
                                start=(j == 0), stop=(j == D_CONV - 1),
                                skip_group_check=True)
                    for i in (1, 0):
                        base = (lp * 2 + i) * 512
                        nc.scalar.activation(xi[b][d][:, base:base + 512], pts[i][:],
                                             AF.Silu, bias=c_cb[:, d:d + 1])
            # x_proj partial: [96, SEQ] = xproj_wT.T @ xi
            xdbl_sb = fp.tile([NX, SEQ], BF16, tag=f"xdbl{b}", name=f"xdbl{b}")
            for lc in range(SEQ // 512):
                pt = fps.tile([128, 512], F32, tag="rrep", name="xproj", bufs=1)
                for d in range(NDT):
                    nc.tensor.matmul(pt[:NX, :], w_xproj[:, d * NX:(d + 1) * NX],
                                     xi[b][d][:, lc * 512:(lc + 1) * 512],
                                     start=(d == 0), stop=(d == NDT - 1))
                nc.vector.tensor_scalar_mul(xdbl_sb[:, lc * 512:(lc + 1) * 512],
                                            pt[:NX, :], 1.0)
            nc.scalar.dma_start(ar_in[b][:], xdbl_sb[:])
            if SIM_NO_COLLECTIVES:
                nc.scalar.dma_start(ar_out[b][:], ar_in[b][:])
            else:
                nc.gpsimd.collective_compute(
                    "AllReduce", OP.add, ins=[ar_in[b][:]], outs=[ar_out[b][:]],
                    replica_groups=GROUPS)

        def dt_batch(b, dp, dps):
            dtlow = dp.tile([DT_RANK, SEQ], BF16, tag="dtlow", name="dtlow")
            nc.scalar.dma_start(dtlow[:], ar_out[b][0:DT_RANK, :])
            for d in range(NDT):
                et = dp.tile([128, SEQ], F32, tag="spexp", name="spexp", bufs=1)
                for lc in range(SEQ // 512):
                    pt = dps.tile([128, 512], F32, tag="dtmm", name="dtmm", bufs=1)
                    nc.tensor.matmul(pt[:], w_dtproj[:, d * 128:(d + 1) * 128],
                                     dtlow[:, lc * 512:(lc + 1) * 512],
                                     start=True, stop=True)
                    nc.scalar.activation(et[:, lc * 512:(lc + 1) * 512], pt[:],
                                         AF.Exp, bias=c_dtb[:, d:d + 1])
                nc.scalar.activation(dt_t[b][d][:], et[:], AF.Ln, bias=1.0)
                nc.vector.tensor_tensor(du[b][d][:], dt_t[b][d][:], xi[b][d][:],
                                        OP.mult)

        def scan_ds(b, ds, sps, sp):
            """Scan for batch b over all d-tiles (n-outer, shared broadcasts)."""
            y_acc = {d: sps.tile([128, SEQ], F32, tag=f"yacc{d}", name=f"yacc{d}",
                                 bufs=1) for d in ds}
            for n in range(D_STATE):
                b_rep = sp.tile([128, SEQ], BF16, tag="brep", name="brep", bufs=2)
                nc.scalar.dma_start(
                    b_rep[:],
                    ar_out[b][DT_RANK + n:DT_RANK + n + 1, :].to_broadcast((128, SEQ)))
                c_rep = sp.tile([128, SEQ], BF16, tag="crep", name="crep", bufs=2)
                nc.scalar.dma_start(
                    c_rep[:],
                    ar_out[b][DT_RANK + D_STATE + n:DT_RANK + D_STATE + n + 1, :]
                    .to_broadcast((128, SEQ)))
                for d in ds:
                    dA = sp.tile([128, SEQ], F32, tag=f"dA{d}", name=f"dA{d}", bufs=1)
                    nc.scalar.activation(dA[:], dt_t[b][d][:], AF.Exp,
                                         scale=c_A[:, d * D_STATE + n:
                                                   d * D_STATE + n + 1])
                    dBu = sp.tile([128, SEQ], BF16, tag=f"dBu{d}", name=f"dBu{d}",
                                  bufs=2)
                    eng = nc.gpsimd if d == 1 else nc.vector
                    eng.tensor_tensor(dBu[:], du[b][d][:], b_rep[:], OP.mult)
                    h = sp.tile([128, SEQ], BF16, tag=f"h{d}", name=f"h{d}", bufs=1)
                    nc.vector.tensor_tensor_scan(h[:], dA[:], dBu[:], 0.0,
                                                 OP.mult, OP.add)
                    hC = sp.tile([128, SEQ], BF16, tag=f"hC{d}", name=f"hC{d}", bufs=3 if d == 0 else 2)
                    eng2 = nc.gpsimd if (d == 0 and n % 2 == 1) else nc.vector
                    eng2.tensor_tensor(hC[:], h[:], c_rep[:], OP.mult)
                    for lc in range(SEQ // 512):
                        nc.tensor.matmul(
                            y_acc[d][:, lc * 512:(lc + 1) * 512], iden_bf[:],
                            hC[:, lc * 512:(lc + 1) * 512],
                            start=(n == 0), stop=False,
                            skip_group_check=True)
            # inject D*u (stop member), then gate with silu(z)
            for d in ds:
                for lc in range(SEQ // 512):
                    nc.tensor.matmul(
                        y_acc[d][:, lc * 512:(lc + 1) * 512],
                        w_ddiag[:, d * 128:(d + 1) * 128],
                        xi[b][d][:, lc * 512:(lc + 1) * 512],
                        start=False, stop=True, skip_group_check=True)
                ygt = sp.tile([128, SEQ], BF16, tag=f"yg{d}", name=f"yg{d}", bufs=1)
                nc.vector.tensor_tensor(ygt[:], y_acc[d][:], siluz[b][d][:], OP.mult)
                for s in range(N_CORES):
                    nc.scalar.dma_start(
                        a2a_in[b][s, d * 128:(d + 1) * 128, :],
                        ygt[:, s * HTOK:(s + 1) * HTOK])

        def a2a_batch(b):
            if SIM_NO_COLLECTIVES:
                nc.scalar.dma_start(a2a_out[b][:], a2a_in[b][:])
            else:
                nc.gpsimd.collective_compute("AllToAll", OP.bypass,
                                             ins=[a2a_in[b][:]],
                                             outs=[a2a_out[b][:]],
                                             replica_groups=GROUPS)

        def oproj_batch(b, op_, ops, w_outproj):
            yf = []
            for kt in range(D_INNER // 128):
                t = op_.tile([128, HTOK], BF16, tag=f"yf{kt}", name=f"yf{kt}", bufs=1)
                nc.sync.dma_start(t[:], a2a_out[b][:].rearrange("s d q -> (s d) q")
                                  [kt * 128:(kt + 1) * 128, :])
                yf.append(t)
            for mtl in range(HTOK // 128):
                mt = b * (HTOK // 128) + mtl
                for nck in range(DIM // 512):
                    pt = ops.tile([128, 512], F32, tag=f"omm{nck}", name="omm", bufs=1)
                    for kt in range(D_INNER // 128):
                        nc.tensor.matmul(
                            pt[:], yf[kt][:, mtl * 128:(mtl + 1) * 128],
                            w_outproj[:, kt * DIM + nck * 512:
                                      kt * DIM + (nck + 1) * 512],
                            start=(kt == 0), stop=(kt == D_INNER // 128 - 1))
                    ot = op_.tile([128, 512], F32, tag="osb", name="osb", bufs=2)
                    nc.scalar.activation(ot[:], pt[:], AF.Copy)
                    nc.gpsimd.dma_start(
                        out_q[mt * 128:(mt + 1) * 128, nck * 512:(nck + 1) * 512],
                        ot[:])

        # ---------- schedule ----------
        front_batch(0)
        nc.sync.dma_start(w_outproj[:].rearrange("p (k m) -> p k m", k=D_INNER // 128),
                          outproj_wT[:].rearrange("(k p) m -> p k m", p=128))
        with tc.tile_pool(name="dt0", bufs=1) as dp0, \
             tc.tile_pool(name="dt0ps", bufs=1, space="PSUM") as dps0:
            dt_batch(0, dp0, dps0)
            front_batch(1)
        fps_pool.__exit__(None, None, None)
        fp_pool.__exit__(None, None, None)

        with tc.tile_pool(name="scanps0", bufs=1, space="PSUM") as sps0, \
             tc.tile_pool(name="scan0", bufs=1) as sp0:
            scan_ds(0, [0, 1], sps0, sp0)

        # inter-scan dip: dt1 + A2A0 + out_proj(b0) all overlap scan(b1) ramp
        with tc.tile_pool(name="dt1", bufs=1) as dp1, \
             tc.tile_pool(name="dt1ps", bufs=1, space="PSUM") as dps1:
            dt_batch(1, dp1, dps1)
            a2a_batch(0)

        with tc.tile_pool(name="oproj", bufs=1) as op_:
            sp1_pool = tc.tile_pool(name="scan1", bufs=1)
            sp1 = sp1_pool.__enter__()
            with tc.tile_pool(name="ops0", bufs=1, space="PSUM") as ops0:
                oproj_batch(0, op_, ops0, w_outproj)
            with tc.tile_pool(name="scanps1", bufs=1, space="PSUM") as sps1:
                scan_ds(1, [0, 1], sps1, sp1)
            sp1_pool.__exit__(None, None, None)
            a2a_batch(1)
            with tc.tile_pool(name="ops1", bufs=1, space="PSUM") as ops1:
                oproj_batch(1, op_, ops1, w_outproj)

    nc.compile()
    _cache["nc"] = nc
    return nc


def _get_runner():
    """Cached shard_map jit over the bass custom call."""
    if "runner" in _cache:
        return _cache["runner"]
    nc = _build()

    import jax
    import concourse.bass2jax as b2j
    from concourse.bass2jax import _bass_exec_p, partition_id_tensor
    from jax.sharding import Mesh, PartitionSpec
    from jax.experimental.shard_map import shard_map

    b2j.install_neuronx_cc_hook()

    partition_name = nc.partition_id_tensor.name if nc.partition_id_tensor else None
    in_names, out_names, out_avals, zero_shapes = [], [], [], []
    for alloc in nc.m.functions[0].allocations:
        if not isinstance(alloc, mybir.MemoryLocationSet):
            continue
        name = alloc.memorylocations[0].name
        if alloc.kind == "ExternalInput":
            if name != partition_name:
                in_names.append(name)
        elif alloc.kind == "ExternalOutput":
            shape = tuple(alloc.tensor_shape)
            dtype = mybir.dt.np(alloc.dtype)
            out_names.append(name)
            out_avals.append(jax.core.ShapedArray(shape, dtype))
            zero_shapes.append((shape, dtype))
    n_params = len(in_names)
    n_outs = len(out_avals)
    all_in_names = list(in_names) + list(out_names)
    if partition_name is not None:
        all_in_names.append(partition_name)

    def _body(*args):
        operands = list(args)
        if partition_name is not None:
            operands.append(partition_id_tensor())
        return tuple(_bass_exec_p.bind(
            *operands, out_avals=tuple(out_avals),
            in_names=tuple(all_in_names), out_names=tuple(out_names),
            lowering_input_output_aliases=(), sim_require_finite=True,
            sim_require_nnan=True, nc=nc))

    devices = jax.devices()[:N_CORES]
    mesh = Mesh(np.asarray(devices), ("core",))
    donate = tuple(range(n_params, n_params + n_outs))
    sharded = jax.jit(
        shard_map(_body, mesh=mesh,
                  in_specs=(PartitionSpec("core"),) * (n_params + n_outs),
                  out_specs=(PartitionSpec("core"),) * n_outs,
                  check_rep=False),
        donate_argnums=donate, keep_unused=True)

    def run(in_maps):
        concat_in = [np.concatenate([np.asarray(in_maps[c][n]) for c in range(N_CORES)],
                                    axis=0) for n in in_names]
        concat_zeros = [np.zeros((N_CORES * s[0], *s[1:]), d) for s, d in zero_shapes]
        out_arrs = sharded(*concat_in, *concat_zeros)
        return [
            {n: np.asarray(out_arrs[i]).reshape(N_CORES, *out_avals[i].shape)[c]
             for i, n in enumerate(out_names)}
            for c in range(N_CORES)
        ]

    _cache["parts"] = (sharded, in_names, out_names, out_avals, zero_shapes, mesh)
    _cache["runner"] = run
    return run


def kernel(hidden_states, residual, norm_weight, norm_bias, in_proj_w, conv_w,
           conv_b, x_proj_w, dt_proj_w, dt_proj_b, A_log, D_param, out_proj_w):
    run = _get_runner()
    f32 = np.float32
    import ml_dtypes
    bf16 = ml_dtypes.bfloat16

    hid = np.asarray(hidden_states, f32)
    res = np.asarray(residual, f32)
    res_sum = hid + res                       # residual output, exact f32
    rs_T_bf = np.ascontiguousarray(np.swapaxes(res_sum, 1, 2)).astype(bf16)
    outproj_wT = np.ascontiguousarray(np.asarray(out_proj_w, f32).T).astype(bf16)

    nw = np.asarray(norm_weight, f32)
    nb = np.asarray(norm_bias, f32)
    W = np.asarray(in_proj_w, f32)
    Wn = W * nw[None, :]                       # fold norm weight
    bias_full = W @ nb                         # fold norm bias
    A_full = -np.exp(np.asarray(A_log, f32))

    in_maps = []
    for g in range(N_CORES):
        dg = slice(g * DG, (g + 1) * DG)
        w_x = Wn[dg.start:dg.stop]
        w_z = Wn[D_INNER + dg.start:D_INNER + dg.stop]
        inproj_wT = np.ascontiguousarray(np.concatenate([w_x, w_z], 0).T)
        b_x = bias_full[dg.start:dg.stop]
        b_z = bias_full[D_INNER + dg.start:D_INNER + dg.stop]
        cw = np.asarray(conv_w[dg], f32)                       # (256, 4)
        diag = np.zeros((D_CONV, NDT, 128, 128), f32)
        for j in range(D_CONV):
            for d in range(NDT):
                np.fill_diagonal(diag[j, d], cw[d * 128:(d + 1) * 128, j])
        ddiag = np.zeros((NDT, 128, 128), f32)
        for d in range(NDT):
            np.fill_diagonal(ddiag[d], np.asarray(D_param[dg], f32)[d * 128:(d + 1) * 128])
        in_maps.append({
            "rs_T": rs_T_bf,
            "inproj_wT": inproj_wT.astype(bf16),
            "inproj_b": np.concatenate([b_x, b_z]).reshape(2 * DG, 1),
            "conv_diag": diag.reshape(D_CONV * NDT * 128, 128).astype(bf16),
            "conv_b": np.asarray(conv_b[dg], f32).reshape(DG, 1),
            "xproj_wT": np.ascontiguousarray(np.asarray(x_proj_w, f32)[:, dg].T).astype(bf16),
            "dtproj_wT": np.ascontiguousarray(np.asarray(dt_proj_w, f32)[dg].T).astype(bf16),
            "dtproj_b": np.asarray(dt_proj_b[dg], f32).reshape(DG, 1),
            "A_g": A_full[dg],
            "d_diag": ddiag.reshape(NDT * 128, 128).astype(bf16),
            "outproj_wT": outproj_wT,
        })

    results = run(in_maps)

    out = np.empty((BATCH, SEQ, DIM), f32)
    for g in range(N_CORES):
        ts = slice(g * HTOK, (g + 1) * HTOK)
        oq = results[g]["out_q"]                  # (QTOK, DIM)
        out[0, ts] = oq[:HTOK]
        out[1, ts] = oq[HTOK:]
    return out, res_sum
